# revision 9
# baseline (speedup 1.0000x reference)
"""MultiHeadDiffAttention Trainium2 Bass kernel.

Strategy: data-parallel over (batch, query-row-chunk). 8 cores, each runs the
same program on different data: core c handles batch b = c // 4 and query rows
[(c % 4) * 512, (c % 4 + 1) * 512). No collectives needed — softmax is over
keys (fully on-core) and LayerNorm is per-token (fully on-core).

Per-core dataflow (all matmuls fp32r: full-rate 4-byte PE mode, ~1.5e-4 rel):
  - host pre-transposes q/k/v to [D, T] layout (pure data movement)
  - qhT[j] = (Wq/8).T @ qT + bq/8          [1024, 512]   (8 j-tiles of 128)
  - per head-pair h (2 doubled heads j0/j1 = partitions 0-63 / 64-127):
      khT_h = Wk_h.T @ kT + bk             [128, 2048]
      vh_h  = vT.T @ Wv_h                  [2048, 128]   (2-pair groups)
      S_j^T = khT_j.T @ qhT_j              [2048, 512]   (K=64 matmuls)
      P_j   = exp(S_j^T)                   (no max-subtraction: |S| <~ 2)
      U_j^T = vh_h.T @ P_j    (PSUM accum over 16 key tiles)   [128, 512]
      den_j = 1^T @ P_j       (PSUM accum)                     [1, 512]
      attnT_h = U0/den0 - lam*U1/den1 + (1-lam)*bv_h           [128, 512]
  - LayerNorm folded into output projection:
      z = attn @ (ln_scale*Wo);  stats via ones-matmuls over attnT
      y = (1-LI)*rstd*(z - mu*colsum(Wo2)) + ((1-LI)*ln_bias@Wo + bo)
"""

import sys

if "/opt/trn_rl_repo" not in sys.path:
    sys.path.insert(0, "/opt/trn_rl_repo")

from contextlib import ExitStack

import numpy as np

B, T, D, H2, DH, M = 2, 2048, 512, 16, 64, 512
HP = 8          # head pairs
C = H2 * DH     # 1024 projected channels
TQ = 512        # query rows per core
NKT = T // 128  # 16 key tiles
LAMBDA_INIT = 0.8 - 0.6 * float(np.exp(-0.3 * 2))
SCALE = 1.0 / float(np.sqrt(DH))
EPS = 1e-6

_CACHE = {}


def _build(lam: float, debug: bool = False):
    import concourse.mybir as mybir
    import concourse.tile as tile
    from concourse import bacc
    from concourse.masks import make_identity

    F32 = mybir.dt.float32
    F32R = mybir.dt.float32r
    AF = mybir.ActivationFunctionType
    OP = mybir.AluOpType

    nc = bacc.Bacc("TRN2", target_bir_lowering=False, debug=False)
    dt_in = lambda n, s: nc.dram_tensor(n, s, F32, kind="ExternalInput").ap()
    qT_d = dt_in("qt", [D, TQ])
    kT_d = dt_in("kt", [D, T])
    vT_d = dt_in("vt", [D, T])
    wq_d = dt_in("wq", [D, C])
    wk_d = dt_in("wk", [D, C])
    wv_d = dt_in("wv", [D, C])
    wo_d = dt_in("wo", [C, M])
    bq_d = dt_in("bq", [128, HP])
    bk_d = dt_in("bk", [128, HP])
    bvl_d = dt_in("bvl", [128, HP])
    srow_d = dt_in("srow", [1, M])
    borow_d = dt_in("borow", [1, M])
    ones_d = dt_in("ones", [128, 1])
    onesr_d = dt_in("onesr", [1, 128])
    negl_d = dt_in("negl", [1, 128])
    y_d = nc.dram_tensor("y", [TQ, M], F32, kind="ExternalOutput").ap()
    dbg = {}
    if debug:
        dbg["qhT0"] = nc.dram_tensor("dbg_qhT0", [128, TQ], F32, kind="ExternalOutput").ap()
        dbg["khT0"] = nc.dram_tensor("dbg_khT0", [128, T], F32, kind="ExternalOutput").ap()
        dbg["vh0"] = nc.dram_tensor("dbg_vh0", [128, 256], F32, kind="ExternalOutput").ap()
        dbg["att"] = nc.dram_tensor("dbg_att", [HP, 128, TQ], F32, kind="ExternalOutput").ap()
        dbg["den"] = nc.dram_tensor("dbg_den", [2 * HP, TQ], F32, kind="ExternalOutput").ap()
        dbg["ab"] = nc.dram_tensor("dbg_ab", [2, TQ], F32, kind="ExternalOutput").ap()
        dbg["e00"] = nc.dram_tensor("dbg_e00", [128, 1024], F32, kind="ExternalOutput").ap()

    with tile.TileContext(nc) as tc, ExitStack() as ctx:
        pp = ctx.enter_context(tc.tile_pool(name="persist", bufs=1))
        khp = ctx.enter_context(tc.tile_pool(name="khp", bufs=1))
        vhp = ctx.enter_context(tc.tile_pool(name="vhp", bufs=1))
        wxp = ctx.enter_context(tc.tile_pool(name="wxp", bufs=1))
        wkp = ctx.enter_context(tc.tile_pool(name="wkp", bufs=2))
        wvp = ctx.enter_context(tc.tile_pool(name="wvp", bufs=2))
        expp = ctx.enter_context(tc.tile_pool(name="expp", bufs=2))
        tmpp = ctx.enter_context(tc.tile_pool(name="tmpp", bufs=4))
        rowp = ctx.enter_context(tc.tile_pool(name="rowp", bufs=3))
        yp = ctx.enter_context(tc.tile_pool(name="yp", bufs=2))
        ps_s = ctx.enter_context(tc.tile_pool(name="ps_s", bufs=2, space="PSUM"))
        ps_u = ctx.enter_context(tc.tile_pool(name="ps_u", bufs=2, space="PSUM"))
        ps_d = ctx.enter_context(tc.tile_pool(name="ps_d", bufs=2, space="PSUM"))

        # ---- constants ----
        ident1 = pp.tile([1, 1], F32, tag="ident1")
        nc.vector.memset(ident1, 1.0)
        ones_col = pp.tile([128, 1], F32R, tag="ones_col")
        nc.sync.dma_start(out=ones_col, in_=ones_d.bitcast(F32R))
        ones_row = pp.tile([1, 128], F32R, tag="ones_row")
        nc.sync.dma_start(out=ones_row, in_=onesr_d.bitcast(F32R))
        negl_row = pp.tile([1, 128], F32R, tag="negl_row")
        nc.sync.dma_start(out=negl_row, in_=negl_d.bitcast(F32R))
        bq_sb = pp.tile([128, HP], F32, tag="bq")
        nc.sync.dma_start(out=bq_sb, in_=bq_d)
        bk_sb = pp.tile([128, HP], F32, tag="bk")
        nc.sync.dma_start(out=bk_sb, in_=bk_d)
        bvl_sb = pp.tile([128, HP], F32, tag="bvl")
        nc.sync.dma_start(out=bvl_sb, in_=bvl_d)
        srow_sb = pp.tile([1, M], F32R, tag="srow")
        nc.sync.dma_start(out=srow_sb, in_=srow_d.bitcast(F32R))
        borow_sb = pp.tile([1, M], F32R, tag="borow")
        nc.sync.dma_start(out=borow_sb, in_=borow_d.bitcast(F32R))
        eps_sb = pp.tile([1, 1], F32, tag="eps")
        nc.vector.memset(eps_sb, EPS)

        # ---- persistent transposed inputs ----
        kT = pp.tile([128, 4, T], F32R, tag="kT")
        nc.sync.dma_start(out=kT, in_=kT_d.rearrange("(a p) t -> p a t", p=128).bitcast(F32R))
        vT = pp.tile([128, 4, T], F32R, tag="vT")
        nc.sync.dma_start(out=vT, in_=vT_d.rearrange("(a p) t -> p a t", p=128).bitcast(F32R))
        qTt = pp.tile([128, 4, TQ], F32R, tag="qT")
        nc.sync.dma_start(out=qTt, in_=qT_d.rearrange("(a p) t -> p a t", p=128).bitcast(F32R))

        # ---- q projection: qhT[j] [128, 512] ----
        wq_sb = wxp.tile([128, 4, C], F32R, tag="wx")
        nc.sync.dma_start(out=wq_sb, in_=wq_d.rearrange("(a p) n -> p a n", p=128).bitcast(F32R))
        qhT = []
        for j in range(HP):
            pq = ps_s.tile([128, TQ], F32, tag="ps_s")
            for a in range(4):
                nc.tensor.matmul(pq, wq_sb[:, a, 128 * j:128 * (j + 1)], qTt[:, a, :],
                                 start=(a == 0), stop=(a == 3))
            t = pp.tile([128, TQ], F32R, tag=f"qhT{j}")
            nc.vector.tensor_scalar_add(t, pq, bq_sb[:, j:j + 1])
            qhT.append(t)
            if debug and j == 0:
                nc.sync.dma_start(out=dbg["qhT0"], in_=t.bitcast(F32))

        # ---- pair loop ----
        attnT = []
        for h in range(HP):
            # vh for a 2-pair group (pairs h, h+1) when h even
            if h % 2 == 0:
                wv_sl = wvp.tile([128, 4, 256], F32R, tag="wv")
                nc.sync.dma_start(
                    out=wv_sl,
                    in_=wv_d.rearrange("(a p) n -> p a n", p=128)[:, :, 256 * (h // 2):256 * (h // 2 + 1)].bitcast(F32R))
                vhg = vhp.tile([128, NKT, 256], F32R, tag="vhg")
                for i in range(NKT):
                    pv = ps_s.tile([128, 256], F32, tag="ps_s")
                    for a in range(4):
                        nc.tensor.matmul(pv, vT[:, a, 128 * i:128 * (i + 1)], wv_sl[:, a, :],
                                         start=(a == 0), stop=(a == 3))
                    nc.vector.tensor_copy(out=vhg[:, i, :], in_=pv)
            vh_h = vhg[:, :, 128 * (h % 2):128 * (h % 2) + 128]

            # khT_h [128, 2048]
            wk_sl = wkp.tile([128, 4, 128], F32R, tag="wk")
            nc.sync.dma_start(
                out=wk_sl,
                in_=wk_d.rearrange("(a p) n -> p a n", p=128)[:, :, 128 * h:128 * (h + 1)].bitcast(F32R))
            khT = khp.tile([128, T], F32R, tag="khT")
            for cch in range(4):
                pk = ps_s.tile([128, 512], F32, tag="ps_s")
                for a in range(4):
                    nc.tensor.matmul(pk, wk_sl[:, a, :], kT[:, a, 512 * cch:512 * (cch + 1)],
                                     start=(a == 0), stop=(a == 3))
                nc.vector.tensor_scalar_add(khT[:, 512 * cch:512 * (cch + 1)], pk, bk_sb[:, h:h + 1])

            # attention: 8 groups of 2 key tiles
            U0 = ps_u.tile([128, TQ], F32, tag="ps_u")
            U1 = ps_u.tile([128, TQ], F32, tag="ps_u")
            den0 = ps_d.tile([1, TQ], F32, tag="ps_d")
            den1 = ps_d.tile([1, TQ], F32, tag="ps_d")
            for g in range(8):
                S0 = ps_s.tile([128, 1024], F32, tag="ps_s")
                S1 = ps_s.tile([128, 1024], F32, tag="ps_s")
                for st in range(2):
                    i = 2 * g + st
                    nc.tensor.matmul(S0[:, 512 * st:512 * (st + 1)],
                                     khT[0:64, 128 * i:128 * (i + 1)], qhT[h][0:64, :],
                                     start=True, stop=True)
                    nc.tensor.matmul(S1[:, 512 * st:512 * (st + 1)],
                                     khT[64:128, 128 * i:128 * (i + 1)], qhT[h][64:128, :],
                                     start=True, stop=True)
                e0 = expp.tile([128, 1024], F32R, tag="exp0")
                e1 = expp.tile([128, 1024], F32R, tag="exp1")
                nc.scalar.activation(out=e0, in_=S0, func=AF.Exp)
                nc.scalar.activation(out=e1, in_=S1, func=AF.Exp)
                if debug and h == 0 and g == 0:
                    nc.sync.dma_start(out=dbg["e00"], in_=e0.bitcast(F32))
                for st in range(2):
                    i = 2 * g + st
                    sl = slice(512 * st, 512 * (st + 1))
                    vsl = vh_h[:, i, :]
                    nc.tensor.matmul(U0, vsl, e0[:, sl], start=(i == 0), stop=(i == NKT - 1))
                    nc.tensor.matmul(U1, vsl, e1[:, sl], start=(i == 0), stop=(i == NKT - 1))
                    nc.tensor.matmul(den0, ones_col, e0[:, sl], start=(i == 0), stop=(i == NKT - 1))
                    nc.tensor.matmul(den1, ones_col, e1[:, sl], start=(i == 0), stop=(i == NKT - 1))

            inv0 = tmpp.tile([1, TQ], F32R, tag="tmp")
            inv1 = tmpp.tile([1, TQ], F32R, tag="tmp")
            with nc.allow_low_precision(reason="f32r rounding of softmax denominators"):
                nc.vector.reciprocal(out=inv0, in_=den0)
                nc.vector.reciprocal(out=inv1, in_=den1)
            pb0 = ps_s.tile([128, TQ], F32, tag="ps_s")
            nc.tensor.matmul(pb0, ones_row, inv0, start=True, stop=True)
            pb1 = ps_s.tile([128, TQ], F32, tag="ps_s")
            nc.tensor.matmul(pb1, negl_row, inv1, start=True, stop=True)
            u0s = tmpp.tile([128, TQ], F32, tag="tmp")
            nc.vector.tensor_copy(out=u0s, in_=U0)
            u1s = tmpp.tile([128, TQ], F32, tag="tmp")
            nc.vector.tensor_copy(out=u1s, in_=U1)
            t1 = tmpp.tile([128, TQ], F32, tag="tmp")
            nc.vector.tensor_tensor(t1, u0s, pb0, OP.mult)
            t2 = tmpp.tile([128, TQ], F32, tag="tmp")
            nc.vector.tensor_tensor(t2, u1s, pb1, OP.mult)
            t3 = tmpp.tile([128, TQ], F32, tag="tmp")
            nc.gpsimd.tensor_tensor(t3, t1, t2, OP.add)
            at = pp.tile([128, TQ], F32R, tag=f"attnT{h}")
            nc.gpsimd.tensor_scalar_add(at, t3, bvl_sb[:, h:h + 1])
            attnT.append(at)
            if debug:
                nc.sync.dma_start(out=dbg["att"][h], in_=at.bitcast(F32))
                dr0 = tmpp.tile([1, TQ], F32, tag="tmp")
                nc.vector.tensor_copy(out=dr0, in_=den0)
                nc.sync.dma_start(out=dbg["den"][2 * h:2 * h + 1], in_=dr0)
                dr1 = tmpp.tile([1, TQ], F32, tag="tmp")
                nc.vector.tensor_copy(out=dr1, in_=den1)
                nc.sync.dma_start(out=dbg["den"][2 * h + 1:2 * h + 2], in_=dr1)
                if h == 0:
                    nc.sync.dma_start(out=dbg["khT0"], in_=khT.bitcast(F32))
                    nc.sync.dma_start(out=dbg["vh0"], in_=vhg[:, 0, :].bitcast(F32))

        # ---- LN stats ----
        pssum = ps_d.tile([1, TQ], F32, tag="ps_d")
        for h in range(HP):
            nc.tensor.matmul(pssum, ones_col, attnT[h], start=(h == 0), stop=(h == HP - 1))
        pssq = ps_d.tile([1, TQ], F32, tag="ps_d")
        for h in range(HP):
            sq = tmpp.tile([128, TQ], F32R, tag="tmp")
            nc.vector.tensor_tensor(sq, attnT[h].bitcast(F32), attnT[h].bitcast(F32), OP.mult)
            nc.tensor.matmul(pssq, ones_col, sq, start=(h == 0), stop=(h == HP - 1))

        mu = rowp.tile([1, TQ], F32, tag="row")
        nc.vector.tensor_scalar_mul(mu, pssum, 1.0 / C)
        e2 = rowp.tile([1, TQ], F32, tag="row")
        nc.vector.tensor_scalar_mul(e2, pssq, 1.0 / C)
        musq = rowp.tile([1, TQ], F32, tag="row")
        nc.vector.tensor_tensor(musq, mu, mu, OP.mult)
        nc.vector.tensor_tensor(e2, e2, musq, OP.subtract)       # var (in-place)
        nc.scalar.activation(out=musq, in_=e2, func=AF.Sqrt, bias=eps_sb)  # std
        nc.vector.reciprocal(out=e2, in_=musq)                   # rstd
        a_row = pp.tile([1, TQ], F32, tag="a_row")
        nc.vector.tensor_scalar_mul(a_row, e2, 1.0 - LAMBDA_INIT)
        # y = a * (attn@Wo2 - mu (x) srow + (1/a) (x) borow); the last two are
        # rank-1 terms folded into the z accumulation so one per-row scale
        # finishes the LayerNorm + bias exactly.
        negmu_row = pp.tile([1, TQ], F32R, tag="negmu")
        nc.vector.tensor_scalar_mul(negmu_row, mu, -1.0)
        inva_row = pp.tile([1, TQ], F32R, tag="inva")
        with nc.allow_low_precision(reason="f32r rounding of 1/a rank-1 bias term"):
            nc.vector.reciprocal(out=inva_row, in_=a_row)
        if debug:
            nc.sync.dma_start(out=dbg["ab"][0:1], in_=a_row)
            nc.sync.dma_start(out=dbg["ab"][1:2], in_=negmu_row.bitcast(F32))

        a_col = pp.tile([128, 4], F32, tag="a_col")
        for t in range(4):
            pt = ps_d.tile([128, 1], F32, tag="ps_d")
            nc.tensor.transpose(pt, a_row[:, 128 * t:128 * (t + 1)], ident1)
            nc.vector.tensor_copy(out=a_col[:, t:t + 1], in_=pt)

        # ---- output projection + fixup ----
        wo_sb = wxp.tile([128, HP, M], F32R, tag="wx")
        nc.sync.dma_start(out=wo_sb, in_=wo_d.rearrange("(a p) m -> p a m", p=128).bitcast(F32R))
        for t in range(4):
            pz = ps_s.tile([128, M], F32, tag="ps_s")
            for h in range(HP):
                nc.tensor.matmul(pz, attnT[h][:, 128 * t:128 * (t + 1)], wo_sb[:, h, :],
                                 start=(h == 0), stop=False)
            nc.tensor.matmul(pz, negmu_row[:, 128 * t:128 * (t + 1)], srow_sb, start=False, stop=False)
            nc.tensor.matmul(pz, inva_row[:, 128 * t:128 * (t + 1)], borow_sb, start=False, stop=True)
            y_sb = yp.tile([128, M], F32, tag="y")
            nc.vector.tensor_scalar_mul(y_sb, pz, a_col[:, t:t + 1])
            nc.sync.dma_start(out=y_d[128 * t:128 * (t + 1), :], in_=y_sb)

    nc.compile()
    return nc


def _prep_inputs(inputs):
    """Host-side prep shared by all cores; returns (lam, shared dict)."""
    f32 = np.float32
    q = np.asarray(inputs["query"], f32)
    k = np.asarray(inputs["key"], f32)
    v = np.asarray(inputs["value"], f32)
    Wq = np.asarray(inputs["Wq"], f32)
    Wk = np.asarray(inputs["Wk"], f32)
    Wv = np.asarray(inputs["Wv"], f32)
    Wo = np.asarray(inputs["Wo"], f32)
    bq = np.asarray(inputs["bq"], f32)
    bk = np.asarray(inputs["bk"], f32)
    bv = np.asarray(inputs["bv"], f32)
    bo = np.asarray(inputs["bo"], f32)
    lq1 = np.asarray(inputs["lq1"], f32)
    lk1 = np.asarray(inputs["lk1"], f32)
    lq2 = np.asarray(inputs["lq2"], f32)
    lk2 = np.asarray(inputs["lk2"], f32)
    ln_scale = np.asarray(inputs["ln_scale"], f32)
    ln_bias = np.asarray(inputs["ln_bias"], f32)

    lam = float(np.exp(np.sum(lq1 * lk1, dtype=f32)) - np.exp(np.sum(lq2 * lk2, dtype=f32)) + LAMBDA_INIT)

    Wo2 = (ln_scale[:, None] * Wo).astype(f32)
    shared = {
        "wq": np.ascontiguousarray(Wq * SCALE),
        "wk": np.ascontiguousarray(Wk),
        "wv": np.ascontiguousarray(Wv),
        "wo": np.ascontiguousarray(Wo2),
        "bq": np.ascontiguousarray((bq * SCALE).reshape(HP, 128).T),
        "bk": np.ascontiguousarray(bk.reshape(HP, 128).T),
        "bvl": np.ascontiguousarray(((1.0 - lam) * bv).reshape(HP, 128).T),
        "srow": np.ascontiguousarray(Wo2.sum(axis=0, dtype=f32)[None, :]),
        "borow": np.ascontiguousarray(
            ((1.0 - LAMBDA_INIT) * (ln_bias @ Wo) + bo)[None, :].astype(f32)),
        "ones": np.ones((128, 1), f32),
        "onesr": np.ones((1, 128), f32),
        "negl": np.full((1, 128), -lam, f32),
    }
    return lam, q, k, v, shared


def kernel(**inputs) -> np.ndarray:
    from concourse import bass_utils

    lam, q, k, v, shared = _prep_inputs(inputs)
    key = round(lam, 6)
    if key not in _CACHE:
        _CACHE[key] = _build(lam)
    nc = _CACHE[key]

    in_maps = []
    for c in range(8):
        b, ch = c // 4, c % 4
        in_maps.append(dict(
            shared,
            qt=np.ascontiguousarray(q[b, ch * TQ:(ch + 1) * TQ, :].T),
            kt=np.ascontiguousarray(k[b].T),
            vt=np.ascontiguousarray(v[b].T),
        ))

    res = bass_utils.run_bass_kernel_spmd(nc, in_maps, core_ids=list(range(8)))
    out = np.empty((B, T, M), np.float32)
    for c in range(8):
        b, ch = c // 4, c % 4
        out[b, ch * TQ:(ch + 1) * TQ, :] = res.results[c]["y"]
    return out


# revision 15
# speedup vs baseline: 1.2677x; 1.2677x over previous
"""MultiHeadDiffAttention Trainium2 Bass kernel.

Strategy: data-parallel over (batch, query-row-chunk). 8 cores, each runs the
same program on different data: core c handles batch b = c // 4 and query rows
[(c % 4) * 512, (c % 4 + 1) * 512). No collectives needed — softmax is over
keys (fully on-core) and LayerNorm is per-token (fully on-core).

Per-core dataflow (all matmuls fp32r: full-rate 4-byte PE mode, ~1.5e-4 rel):
  - host pre-transposes q/k/v to [D, T] layout (pure data movement)
  - qhT[j] = (Wq/8).T @ qT + bq/8          [1024, 512]   (8 j-tiles of 128)
  - per head-pair h (2 doubled heads j0/j1 = partitions 0-63 / 64-127):
      khT_h = Wk_h.T @ kT + bk             [128, 2048]
      vh_h  = vT.T @ Wv_h                  [2048, 128]   (2-pair groups)
      S_j^T = khT_j.T @ qhT_j              [2048, 512]   (K=64 matmuls)
      P_j   = exp(S_j^T)                   (no max-subtraction: |S| <~ 2)
      U_j^T = vh_h.T @ P_j    (PSUM accum over 16 key tiles)   [128, 512]
      den_j = 1^T @ P_j       (PSUM accum)                     [1, 512]
      attnT_h = U0/den0 - lam*U1/den1 + (1-lam)*bv_h           [128, 512]
  - LayerNorm folded into output projection:
      z = attn @ (ln_scale*Wo);  stats via ones-matmuls over attnT
      y = (1-LI)*rstd*(z - mu*colsum(Wo2)) + ((1-LI)*ln_bias@Wo + bo)
"""

import sys

if "/opt/trn_rl_repo" not in sys.path:
    sys.path.insert(0, "/opt/trn_rl_repo")

from contextlib import ExitStack

import numpy as np

B, T, D, H2, DH, M = 2, 2048, 512, 16, 64, 512
HP = 8          # head pairs
C = H2 * DH     # 1024 projected channels
TQ = 512        # query rows per core
NKT = T // 128  # 16 key tiles
LAMBDA_INIT = 0.8 - 0.6 * float(np.exp(-0.3 * 2))
SCALE = 1.0 / float(np.sqrt(DH))
EPS = 1e-6

_CACHE = {}


def _build(lam: float, debug: bool = False):
    import concourse.mybir as mybir
    import concourse.tile as tile
    from concourse import bacc
    from concourse.masks import make_identity

    F32 = mybir.dt.float32
    F32R = mybir.dt.float32r
    F16 = mybir.dt.float16
    AF = mybir.ActivationFunctionType
    OP = mybir.AluOpType

    nc = bacc.Bacc("TRN2", target_bir_lowering=False, debug=False)
    dt_in = lambda n, s: nc.dram_tensor(n, s, F32, kind="ExternalInput").ap()
    dt16 = lambda n, s: nc.dram_tensor(n, s, F16, kind="ExternalInput").ap()
    qT_d = dt16("qt", [D, TQ])
    kT_d = dt16("kt", [D, T])
    vT_d = dt16("vt", [D, T])
    wq_d = dt16("wq", [D, C])
    wk_d = dt16("wk", [D, C])
    wv_d = dt16("wv", [D, C])
    wo_d = dt16("wo", [C, M])
    bq_d = dt_in("bq", [128, HP])
    bk_d = dt_in("bk", [128, HP])
    bvl_d = dt_in("bvl", [128, HP])
    srow_d = dt16("srow", [1, M])
    borow_d = dt16("borow", [1, M])
    ones_d = dt16("ones", [128, 1])
    onesr_d = dt16("onesr", [1, 128])
    negl_d = dt16("negl", [1, 128])
    y_d = nc.dram_tensor("y", [TQ, M], F32, kind="ExternalOutput").ap()
    dbg = {}
    if debug:
        dbg["qhT0"] = nc.dram_tensor("dbg_qhT0", [128, TQ], F16, kind="ExternalOutput").ap()
        dbg["khT0"] = nc.dram_tensor("dbg_khT0", [128, T], F16, kind="ExternalOutput").ap()
        dbg["vh0"] = nc.dram_tensor("dbg_vh0", [128, 256], F16, kind="ExternalOutput").ap()
        dbg["att"] = nc.dram_tensor("dbg_att", [HP, 128, TQ], F16, kind="ExternalOutput").ap()
        dbg["den"] = nc.dram_tensor("dbg_den", [2 * HP, TQ], F32, kind="ExternalOutput").ap()
        dbg["ab"] = nc.dram_tensor("dbg_ab", [2, TQ], F32, kind="ExternalOutput").ap()
        dbg["e00"] = nc.dram_tensor("dbg_e00", [128, 1024], F16, kind="ExternalOutput").ap()

    with tile.TileContext(nc) as tc, ExitStack() as ctx:
        pp = ctx.enter_context(tc.tile_pool(name="persist", bufs=1))
        khp = ctx.enter_context(tc.tile_pool(name="khp", bufs=1))
        vhp = ctx.enter_context(tc.tile_pool(name="vhp", bufs=1))
        wxp = ctx.enter_context(tc.tile_pool(name="wxp", bufs=1))
        wkp = ctx.enter_context(tc.tile_pool(name="wkp", bufs=2))
        wvp = ctx.enter_context(tc.tile_pool(name="wvp", bufs=2))
        expp = ctx.enter_context(tc.tile_pool(name="expp", bufs=2))
        tmpp = ctx.enter_context(tc.tile_pool(name="tmpp", bufs=4))
        rowp = ctx.enter_context(tc.tile_pool(name="rowp", bufs=3))
        yp = ctx.enter_context(tc.tile_pool(name="yp", bufs=2))
        ps_s = ctx.enter_context(tc.tile_pool(name="ps_s", bufs=2, space="PSUM"))
        ps_u = ctx.enter_context(tc.tile_pool(name="ps_u", bufs=2, space="PSUM"))
        ps_d = ctx.enter_context(tc.tile_pool(name="ps_d", bufs=2, space="PSUM"))

        # ---- constants ----
        ident1 = pp.tile([1, 1], F32, tag="ident1")
        nc.vector.memset(ident1, 1.0)
        ones_col = pp.tile([128, 1], F16, tag="ones_col")
        nc.sync.dma_start(out=ones_col, in_=ones_d)
        ones_row = pp.tile([1, 128], F16, tag="ones_row")
        nc.sync.dma_start(out=ones_row, in_=onesr_d)
        negl_row = pp.tile([1, 128], F16, tag="negl_row")
        nc.sync.dma_start(out=negl_row, in_=negl_d)
        bq_sb = pp.tile([128, HP], F32, tag="bq")
        nc.sync.dma_start(out=bq_sb, in_=bq_d)
        bk_sb = pp.tile([128, HP], F32, tag="bk")
        nc.sync.dma_start(out=bk_sb, in_=bk_d)
        bvl_sb = pp.tile([128, HP], F32, tag="bvl")
        nc.sync.dma_start(out=bvl_sb, in_=bvl_d)
        srow_sb = pp.tile([1, M], F16, tag="srow")
        nc.sync.dma_start(out=srow_sb, in_=srow_d)
        borow_sb = pp.tile([1, M], F16, tag="borow")
        nc.sync.dma_start(out=borow_sb, in_=borow_d)
        eps_sb = pp.tile([1, 1], F32, tag="eps")
        nc.vector.memset(eps_sb, EPS)

        # ---- persistent transposed inputs ----
        kT = pp.tile([128, 4, T], F16, tag="kT")
        nc.sync.dma_start(out=kT, in_=kT_d.rearrange("(a p) t -> p a t", p=128))
        vT = pp.tile([128, 4, T], F16, tag="vT")
        nc.sync.dma_start(out=vT, in_=vT_d.rearrange("(a p) t -> p a t", p=128))
        qTt = pp.tile([128, 4, TQ], F16, tag="qT")
        nc.sync.dma_start(out=qTt, in_=qT_d.rearrange("(a p) t -> p a t", p=128))

        # ---- q projection: qhT[j] [128, 512] ----
        wq_sb = wxp.tile([128, 4, C], F16, tag="wx")
        nc.sync.dma_start(out=wq_sb, in_=wq_d.rearrange("(a p) n -> p a n", p=128))
        qhT = []
        for j in range(HP):
            pq = ps_s.tile([128, TQ], F32, tag="ps_s")
            for a in range(4):
                nc.tensor.matmul(pq, wq_sb[:, a, 128 * j:128 * (j + 1)], qTt[:, a, :],
                                 start=(a == 0), stop=(a == 3))
            t = pp.tile([128, TQ], F16, tag=f"qhT{j}")
            nc.vector.tensor_scalar_add(t, pq, bq_sb[:, j:j + 1])
            qhT.append(t)
            if debug and j == 0:
                nc.sync.dma_start(out=dbg["qhT0"], in_=t.bitcast(F32))

        # ---- pair loop ----
        attnT = []
        for h in range(HP):
            # vh for a 2-pair group (pairs h, h+1) when h even
            if h % 2 == 0:
                wv_sl = wvp.tile([128, 4, 256], F16, tag="wv")
                nc.sync.dma_start(
                    out=wv_sl,
                    in_=wv_d.rearrange("(a p) n -> p a n", p=128)[:, :, 256 * (h // 2):256 * (h // 2 + 1)])
                vhg = vhp.tile([128, NKT, 256], F16, tag="vhg")
                for i in range(NKT):
                    pv = ps_s.tile([128, 256], F32, tag="ps_s")
                    for a in range(4):
                        nc.tensor.matmul(pv, vT[:, a, 128 * i:128 * (i + 1)], wv_sl[:, a, :],
                                         start=(a == 0), stop=(a == 3))
                    nc.vector.tensor_copy(out=vhg[:, i, :], in_=pv)
            vh_h = vhg[:, :, 128 * (h % 2):128 * (h % 2) + 128]

            # khT_h [128, 2048]
            wk_sl = wkp.tile([128, 4, 128], F16, tag="wk")
            nc.sync.dma_start(
                out=wk_sl,
                in_=wk_d.rearrange("(a p) n -> p a n", p=128)[:, :, 128 * h:128 * (h + 1)])
            khT = khp.tile([128, T], F16, tag="khT")
            for cch in range(4):
                pk = ps_s.tile([128, 512], F32, tag="ps_s")
                for a in range(4):
                    nc.tensor.matmul(pk, wk_sl[:, a, :], kT[:, a, 512 * cch:512 * (cch + 1)],
                                     start=(a == 0), stop=(a == 3))
                nc.vector.tensor_scalar_add(khT[:, 512 * cch:512 * (cch + 1)], pk, bk_sb[:, h:h + 1])

            # attention: 8 groups of 2 key tiles
            U0 = ps_u.tile([128, TQ], F32, tag="ps_u")
            U1 = ps_u.tile([128, TQ], F32, tag="ps_u")
            den0 = ps_d.tile([128, TQ], F32, tag="ps_d")
            den1 = ps_d.tile([128, TQ], F32, tag="ps_d")
            nc.vector.memset(den0, 0.0)
            nc.vector.memset(den1, 0.0)
            for g in range(8):
                S0 = ps_s.tile([128, 1024], F32, tag="ps_s")
                S1 = ps_s.tile([128, 1024], F32, tag="ps_s")
                for st in range(2):
                    i = 2 * g + st
                    nc.tensor.matmul(S0[:, 512 * st:512 * (st + 1)],
                                     khT[0:64, 128 * i:128 * (i + 1)], qhT[h][0:64, :],
                                     start=True, stop=True)
                    nc.tensor.matmul(S1[:, 512 * st:512 * (st + 1)],
                                     khT[64:128, 128 * i:128 * (i + 1)], qhT[h][64:128, :],
                                     start=True, stop=True)
                e0 = expp.tile([128, 1024], F16, tag="exp0")
                e1 = expp.tile([128, 1024], F16, tag="exp1")
                nc.scalar.activation(out=e0, in_=S0, func=AF.Exp)
                nc.scalar.activation(out=e1, in_=S1, func=AF.Exp)
                if debug and h == 0 and g == 0:
                    nc.sync.dma_start(out=dbg["e00"], in_=e0)
                for st in range(2):
                    i = 2 * g + st
                    sl = slice(512 * st, 512 * (st + 1))
                    vsl = vh_h[:, i, :]
                    nc.tensor.matmul(U0, vsl, e0[:, sl], start=(i == 0), stop=(i == NKT - 1))
                    nc.tensor.matmul(U1, vsl, e1[:, sl], start=(i == 0), stop=(i == NKT - 1))
                    # softmax denominators: 4-way column-group packing so the
                    # M=1 matmuls run concurrently in separate 32-col strips
                    cg = i % 4
                    nc.tensor.matmul(den0[32 * cg:32 * cg + 1, :], ones_col, e0[:, sl],
                                     start=(i < 4), stop=(i >= NKT - 4),
                                     tile_position=(0, 32 * cg))
                    nc.tensor.matmul(den1[32 * cg:32 * cg + 1, :], ones_col, e1[:, sl],
                                     start=(i < 4), stop=(i >= NKT - 4),
                                     tile_position=(0, 32 * cg))

            invs = []
            for dps in (den0, den1):
                # rows 0/32/64/96 hold col-group partials, the rest are 0 from
                # the memset — one ones-matmul sums all 128 rows into [1, TQ]
                densb = tmpp.tile([128, TQ], F16, tag="tmp")
                nc.vector.tensor_copy(out=densb, in_=dps)
                dtot = ps_d.tile([1, TQ], F32, tag="ps_d")
                nc.tensor.matmul(dtot, ones_col, densb, start=True, stop=True)
                invf = tmpp.tile([1, TQ], F32, tag="tmp")
                scr = tmpp.tile([1, TQ], F32, tag="tmp")
                nc.vector.reciprocal_approx_accurate(out=invf, in_=dtot, scratch=scr)
                inv16 = tmpp.tile([1, TQ], F16, tag="tmp")
                nc.vector.tensor_copy(out=inv16, in_=invf)
                invs.append(inv16)
            inv0, inv1 = invs
            pb0 = ps_s.tile([128, TQ], F32, tag="ps_s")
            nc.tensor.matmul(pb0, ones_row, inv0, start=True, stop=True)
            pb1 = ps_s.tile([128, TQ], F32, tag="ps_s")
            nc.tensor.matmul(pb1, negl_row, inv1, start=True, stop=True)
            u0s = tmpp.tile([128, TQ], F32, tag="tmp")
            nc.vector.tensor_copy(out=u0s, in_=U0)
            u1s = tmpp.tile([128, TQ], F32, tag="tmp")
            nc.vector.tensor_copy(out=u1s, in_=U1)
            t1 = tmpp.tile([128, TQ], F32, tag="tmp")
            nc.vector.tensor_tensor(t1, u0s, pb0, OP.mult)
            t2 = tmpp.tile([128, TQ], F32, tag="tmp")
            nc.vector.tensor_tensor(t2, u1s, pb1, OP.mult)
            t3 = tmpp.tile([128, TQ], F32, tag="tmp")
            nc.gpsimd.tensor_tensor(t3, t1, t2, OP.add)
            at = pp.tile([128, TQ], F16, tag=f"attnT{h}")
            nc.vector.tensor_scalar_add(at, t3, bvl_sb[:, h:h + 1])
            attnT.append(at)
            if debug:
                nc.sync.dma_start(out=dbg["att"][h], in_=at)
                if h == 0:
                    nc.sync.dma_start(out=dbg["khT0"], in_=khT)
                    nc.sync.dma_start(out=dbg["vh0"], in_=vhg[:, 0, :])

        # ---- LN stats ----
        pssum = ps_d.tile([1, TQ], F32, tag="ps_d")
        for h in range(HP):
            nc.tensor.matmul(pssum, ones_col, attnT[h], start=(h == 0), stop=(h == HP - 1))
        pssq = ps_d.tile([1, TQ], F32, tag="ps_d")
        for h in range(HP):
            sq = tmpp.tile([128, TQ], F16, tag="tmp")
            nc.vector.tensor_tensor(sq, attnT[h], attnT[h], OP.mult)
            nc.tensor.matmul(pssq, ones_col, sq, start=(h == 0), stop=(h == HP - 1))

        mu = rowp.tile([1, TQ], F32, tag="row")
        nc.vector.tensor_scalar_mul(mu, pssum, 1.0 / C)
        e2 = rowp.tile([1, TQ], F32, tag="row")
        nc.vector.tensor_scalar_mul(e2, pssq, 1.0 / C)
        musq = rowp.tile([1, TQ], F32, tag="row")
        nc.vector.tensor_tensor(musq, mu, mu, OP.mult)
        nc.vector.tensor_tensor(e2, e2, musq, OP.subtract)       # var (in-place)
        nc.scalar.activation(out=musq, in_=e2, func=AF.Sqrt, bias=eps_sb)  # std
        nc.vector.reciprocal(out=e2, in_=musq)                   # rstd
        a_row = pp.tile([1, TQ], F32, tag="a_row")
        nc.vector.tensor_scalar_mul(a_row, e2, 1.0 - LAMBDA_INIT)
        # y = a * (attn@Wo2 - mu (x) srow + (1/a) (x) borow); the last two are
        # rank-1 terms folded into the z accumulation so one per-row scale
        # finishes the LayerNorm + bias exactly.
        negmu_row = pp.tile([1, TQ], F16, tag="negmu")
        nc.vector.tensor_scalar_mul(negmu_row, mu, -1.0)
        inva_f = rowp.tile([1, TQ], F32, tag="row")
        scr_f = rowp.tile([1, TQ], F32, tag="row")
        nc.vector.reciprocal_approx_accurate(out=inva_f, in_=a_row, scratch=scr_f)
        inva_row = pp.tile([1, TQ], F16, tag="inva")
        nc.vector.tensor_copy(out=inva_row, in_=inva_f)
        if debug:
            nc.sync.dma_start(out=dbg["ab"][0:1], in_=a_row)
            nc.sync.dma_start(out=dbg["ab"][1:2], in_=a_row)

        a_col = pp.tile([128, 4], F32, tag="a_col")
        for t in range(4):
            pt = ps_d.tile([128, 1], F32, tag="ps_d")
            nc.tensor.transpose(pt, a_row[:, 128 * t:128 * (t + 1)], ident1)
            nc.vector.tensor_copy(out=a_col[:, t:t + 1], in_=pt)

        # ---- output projection + fixup ----
        wo_sb = wxp.tile([128, HP, M], F16, tag="wx")
        nc.sync.dma_start(out=wo_sb, in_=wo_d.rearrange("(a p) m -> p a m", p=128))
        for t in range(4):
            pz = ps_s.tile([128, M], F32, tag="ps_s")
            for h in range(HP):
                nc.tensor.matmul(pz, attnT[h][:, 128 * t:128 * (t + 1)], wo_sb[:, h, :],
                                 start=(h == 0), stop=False)
            nc.tensor.matmul(pz, negmu_row[:, 128 * t:128 * (t + 1)], srow_sb, start=False, stop=False)
            nc.tensor.matmul(pz, inva_row[:, 128 * t:128 * (t + 1)], borow_sb, start=False, stop=True)
            y_sb = yp.tile([128, M], F32, tag="y")
            nc.vector.tensor_scalar_mul(y_sb, pz, a_col[:, t:t + 1])
            nc.sync.dma_start(out=y_d[128 * t:128 * (t + 1), :], in_=y_sb)

    nc.compile()
    return nc


def _prep_inputs(inputs):
    """Host-side prep shared by all cores; returns (lam, shared dict)."""
    f32 = np.float32
    q = np.asarray(inputs["query"], f32)
    k = np.asarray(inputs["key"], f32)
    v = np.asarray(inputs["value"], f32)
    Wq = np.asarray(inputs["Wq"], f32)
    Wk = np.asarray(inputs["Wk"], f32)
    Wv = np.asarray(inputs["Wv"], f32)
    Wo = np.asarray(inputs["Wo"], f32)
    bq = np.asarray(inputs["bq"], f32)
    bk = np.asarray(inputs["bk"], f32)
    bv = np.asarray(inputs["bv"], f32)
    bo = np.asarray(inputs["bo"], f32)
    lq1 = np.asarray(inputs["lq1"], f32)
    lk1 = np.asarray(inputs["lk1"], f32)
    lq2 = np.asarray(inputs["lq2"], f32)
    lk2 = np.asarray(inputs["lk2"], f32)
    ln_scale = np.asarray(inputs["ln_scale"], f32)
    ln_bias = np.asarray(inputs["ln_bias"], f32)

    lam = float(np.exp(np.sum(lq1 * lk1, dtype=f32)) - np.exp(np.sum(lq2 * lk2, dtype=f32)) + LAMBDA_INIT)

    f16 = np.float16
    Wo2 = (ln_scale[:, None] * Wo).astype(f32)
    shared = {
        "wq": np.ascontiguousarray((Wq * SCALE).astype(f16)),
        "wk": np.ascontiguousarray(Wk.astype(f16)),
        "wv": np.ascontiguousarray(Wv.astype(f16)),
        "wo": np.ascontiguousarray(Wo2.astype(f16)),
        "bq": np.ascontiguousarray((bq * SCALE).reshape(HP, 128).T),
        "bk": np.ascontiguousarray(bk.reshape(HP, 128).T),
        "bvl": np.ascontiguousarray(((1.0 - lam) * bv).reshape(HP, 128).T),
        "srow": np.ascontiguousarray(Wo2.sum(axis=0, dtype=f32)[None, :].astype(f16)),
        "borow": np.ascontiguousarray(
            ((1.0 - LAMBDA_INIT) * (ln_bias @ Wo) + bo)[None, :].astype(f16)),
        "ones": np.ones((128, 1), f16),
        "onesr": np.ones((1, 128), f16),
        "negl": np.full((1, 128), -lam, f16),
    }
    return lam, q, k, v, shared


def _make_in_maps(q, k, v, shared):
    in_maps = []
    for c in range(8):
        b, ch = c // 4, c % 4
        in_maps.append(dict(
            shared,
            qt=np.ascontiguousarray(q[b, ch * TQ:(ch + 1) * TQ, :].T.astype(np.float16)),
            kt=np.ascontiguousarray(k[b].T.astype(np.float16)),
            vt=np.ascontiguousarray(v[b].T.astype(np.float16)),
        ))
    return in_maps


def kernel(**inputs) -> np.ndarray:
    from concourse import bass_utils

    lam, q, k, v, shared = _prep_inputs(inputs)
    key = round(lam, 6)
    if key not in _CACHE:
        _CACHE[key] = _build(lam)
    nc = _CACHE[key]

    in_maps = _make_in_maps(q, k, v, shared)
    res = bass_utils.run_bass_kernel_spmd(nc, in_maps, core_ids=list(range(8)))
    out = np.empty((B, T, M), np.float32)
    for c in range(8):
        b, ch = c // 4, c % 4
        out[b, ch * TQ:(ch + 1) * TQ, :] = res.results[c]["y"]
    return out


# revision 16
# speedup vs baseline: 1.3279x; 1.0475x over previous
"""MultiHeadDiffAttention Trainium2 Bass kernel.

Strategy: data-parallel over (batch, query-row-chunk). 8 cores, each runs the
same program on different data: core c handles batch b = c // 4 and query rows
[(c % 4) * 512, (c % 4 + 1) * 512). No collectives needed — softmax is over
keys (fully on-core) and LayerNorm is per-token (fully on-core).

Per-core dataflow (all matmuls fp32r: full-rate 4-byte PE mode, ~1.5e-4 rel):
  - host pre-transposes q/k/v to [D, T] layout (pure data movement)
  - qhT[j] = (Wq/8).T @ qT + bq/8          [1024, 512]   (8 j-tiles of 128)
  - per head-pair h (2 doubled heads j0/j1 = partitions 0-63 / 64-127):
      khT_h = Wk_h.T @ kT + bk             [128, 2048]
      vh_h  = vT.T @ Wv_h                  [2048, 128]   (2-pair groups)
      S_j^T = khT_j.T @ qhT_j              [2048, 512]   (K=64 matmuls)
      P_j   = exp(S_j^T)                   (no max-subtraction: |S| <~ 2)
      U_j^T = vh_h.T @ P_j    (PSUM accum over 16 key tiles)   [128, 512]
      den_j = 1^T @ P_j       (PSUM accum)                     [1, 512]
      attnT_h = U0/den0 - lam*U1/den1 + (1-lam)*bv_h           [128, 512]
  - LayerNorm folded into output projection:
      z = attn @ (ln_scale*Wo);  stats via ones-matmuls over attnT
      y = (1-LI)*rstd*(z - mu*colsum(Wo2)) + ((1-LI)*ln_bias@Wo + bo)
"""

import sys

if "/opt/trn_rl_repo" not in sys.path:
    sys.path.insert(0, "/opt/trn_rl_repo")

from contextlib import ExitStack

import numpy as np

B, T, D, H2, DH, M = 2, 2048, 512, 16, 64, 512
HP = 8          # head pairs
C = H2 * DH     # 1024 projected channels
TQ = 512        # query rows per core
NKT = T // 128  # 16 key tiles
LAMBDA_INIT = 0.8 - 0.6 * float(np.exp(-0.3 * 2))
SCALE = 1.0 / float(np.sqrt(DH))
EPS = 1e-6

_CACHE = {}


def _build(lam: float, debug: bool = False):
    import concourse.mybir as mybir
    import concourse.tile as tile
    from concourse import bacc
    from concourse.masks import make_identity

    F32 = mybir.dt.float32
    F32R = mybir.dt.float32r
    F16 = mybir.dt.float16
    AF = mybir.ActivationFunctionType
    OP = mybir.AluOpType

    nc = bacc.Bacc("TRN2", target_bir_lowering=False, debug=False)
    dt_in = lambda n, s: nc.dram_tensor(n, s, F32, kind="ExternalInput").ap()
    dt16 = lambda n, s: nc.dram_tensor(n, s, F16, kind="ExternalInput").ap()
    qT_d = dt16("qt", [D, TQ])
    kT_d = dt16("kt", [D, T])
    vT_d = dt16("vt", [D, T])
    wq_d = dt16("wq", [D, C])
    wk_d = dt16("wk", [D, C])
    wv_d = dt16("wv", [D, C])
    wo_d = dt16("wo", [C, M])
    bq_d = dt_in("bq", [128, HP])
    bk_d = dt_in("bk", [128, HP])
    bvl_d = dt_in("bvl", [128, HP])
    srow_d = dt16("srow", [1, M])
    borow_d = dt16("borow", [1, M])
    ones_d = dt16("ones", [128, 1])
    onesr_d = dt16("onesr", [1, 128])
    negl_d = dt16("negl", [1, 128])
    y_d = nc.dram_tensor("y", [TQ, M], F32, kind="ExternalOutput").ap()
    dbg = {}
    if debug:
        dbg["qhT0"] = nc.dram_tensor("dbg_qhT0", [128, TQ], F16, kind="ExternalOutput").ap()
        dbg["khT0"] = nc.dram_tensor("dbg_khT0", [128, T], F16, kind="ExternalOutput").ap()
        dbg["vh0"] = nc.dram_tensor("dbg_vh0", [128, 256], F16, kind="ExternalOutput").ap()
        dbg["att"] = nc.dram_tensor("dbg_att", [HP, 128, TQ], F16, kind="ExternalOutput").ap()
        dbg["den"] = nc.dram_tensor("dbg_den", [2 * HP, TQ], F32, kind="ExternalOutput").ap()
        dbg["ab"] = nc.dram_tensor("dbg_ab", [2, TQ], F32, kind="ExternalOutput").ap()
        dbg["e00"] = nc.dram_tensor("dbg_e00", [128, 1024], F16, kind="ExternalOutput").ap()

    with tile.TileContext(nc) as tc, ExitStack() as ctx:
        pp = ctx.enter_context(tc.tile_pool(name="persist", bufs=1))
        khp = ctx.enter_context(tc.tile_pool(name="khp", bufs=2))
        vhp = ctx.enter_context(tc.tile_pool(name="vhp", bufs=2))
        wxp = ctx.enter_context(tc.tile_pool(name="wxp", bufs=1))
        wkp = ctx.enter_context(tc.tile_pool(name="wkp", bufs=2))
        wvp = ctx.enter_context(tc.tile_pool(name="wvp", bufs=2))
        expp = ctx.enter_context(tc.tile_pool(name="expp", bufs=3))
        tmpp = ctx.enter_context(tc.tile_pool(name="tmpp", bufs=6))
        rowp = ctx.enter_context(tc.tile_pool(name="rowp", bufs=3))
        yp = ctx.enter_context(tc.tile_pool(name="yp", bufs=2))
        ps_s = ctx.enter_context(tc.tile_pool(name="ps_s", bufs=4, space="PSUM"))
        ps_ud = ctx.enter_context(tc.tile_pool(name="ps_ud", bufs=4, space="PSUM"))

        # ---- constants ----
        ident1 = pp.tile([1, 1], F32, tag="ident1")
        nc.vector.memset(ident1, 1.0)
        ones_col = pp.tile([128, 1], F16, tag="ones_col")
        nc.sync.dma_start(out=ones_col, in_=ones_d)
        ones_row = pp.tile([1, 128], F16, tag="ones_row")
        nc.sync.dma_start(out=ones_row, in_=onesr_d)
        negl_row = pp.tile([1, 128], F16, tag="negl_row")
        nc.sync.dma_start(out=negl_row, in_=negl_d)
        bq_sb = pp.tile([128, HP], F32, tag="bq")
        nc.sync.dma_start(out=bq_sb, in_=bq_d)
        bk_sb = pp.tile([128, HP], F32, tag="bk")
        nc.sync.dma_start(out=bk_sb, in_=bk_d)
        bvl_sb = pp.tile([128, HP], F32, tag="bvl")
        nc.sync.dma_start(out=bvl_sb, in_=bvl_d)
        srow_sb = pp.tile([1, M], F16, tag="srow")
        nc.sync.dma_start(out=srow_sb, in_=srow_d)
        borow_sb = pp.tile([1, M], F16, tag="borow")
        nc.sync.dma_start(out=borow_sb, in_=borow_d)
        eps_sb = pp.tile([1, 1], F32, tag="eps")
        nc.vector.memset(eps_sb, EPS)

        # ---- persistent transposed inputs (q first: unblocks first matmuls) ----
        qTt = pp.tile([128, 4, TQ], F16, tag="qT")
        nc.sync.dma_start(out=qTt, in_=qT_d.rearrange("(a p) t -> p a t", p=128))

        # ---- q projection: qhT[j] [128, 512] ----
        wq_sb = wxp.tile([128, 4, C], F16, tag="wx")
        nc.sync.dma_start(out=wq_sb, in_=wq_d.rearrange("(a p) n -> p a n", p=128))
        qhT = []
        for j in range(HP):
            pq = ps_s.tile([128, TQ], F32, tag="ps_s")
            for a in range(4):
                nc.tensor.matmul(pq, wq_sb[:, a, 128 * j:128 * (j + 1)], qTt[:, a, :],
                                 start=(a == 0), stop=(a == 3))
            t = pp.tile([128, TQ], F16, tag=f"qhT{j}")
            nc.vector.tensor_scalar_add(t, pq, bq_sb[:, j:j + 1])
            qhT.append(t)
            if debug and j == 0:
                nc.sync.dma_start(out=dbg["qhT0"], in_=t.bitcast(F32))

        kT = pp.tile([128, 4, T], F16, tag="kT")
        for a in range(4):
            nc.sync.dma_start(out=kT[:, a, :], in_=kT_d.rearrange("(a p) t -> p a t", p=128)[:, a, :])
        vT = pp.tile([128, 4, T], F16, tag="vT")
        for a in range(4):
            nc.sync.dma_start(out=vT[:, a, :], in_=vT_d.rearrange("(a p) t -> p a t", p=128)[:, a, :])

        # ---- pair loop ----
        attnT = []
        for h in range(HP):
            # vh for a 2-pair group (pairs h, h+1) when h even
            if h % 2 == 0:
                wv_sl = wvp.tile([128, 4, 256], F16, tag="wv")
                nc.sync.dma_start(
                    out=wv_sl,
                    in_=wv_d.rearrange("(a p) n -> p a n", p=128)[:, :, 256 * (h // 2):256 * (h // 2 + 1)])
                vhg = vhp.tile([128, NKT, 256], F16, tag="vhg")
                for i in range(NKT):
                    pv = ps_s.tile([128, 256], F32, tag="ps_s")
                    for a in range(4):
                        nc.tensor.matmul(pv, vT[:, a, 128 * i:128 * (i + 1)], wv_sl[:, a, :],
                                         start=(a == 0), stop=(a == 3))
                    nc.vector.tensor_copy(out=vhg[:, i, :], in_=pv)
            vh_h = vhg[:, :, 128 * (h % 2):128 * (h % 2) + 128]

            # khT_h [128, 2048]
            wk_sl = wkp.tile([128, 4, 128], F16, tag="wk")
            nc.sync.dma_start(
                out=wk_sl,
                in_=wk_d.rearrange("(a p) n -> p a n", p=128)[:, :, 128 * h:128 * (h + 1)])
            khT = khp.tile([128, T], F16, tag="khT")
            for cch in range(4):
                pk = ps_s.tile([128, 512], F32, tag="ps_s")
                for a in range(4):
                    nc.tensor.matmul(pk, wk_sl[:, a, :], kT[:, a, 512 * cch:512 * (cch + 1)],
                                     start=(a == 0), stop=(a == 3))
                nc.vector.tensor_scalar_add(khT[:, 512 * cch:512 * (cch + 1)], pk, bk_sb[:, h:h + 1])

            # attention: 8 groups of 2 key tiles
            U0 = ps_ud.tile([128, TQ], F32, tag="ps_ud")
            U1 = ps_ud.tile([128, TQ], F32, tag="ps_ud")
            den0 = ps_ud.tile([1, TQ], F32, tag="ps_ud")
            den1 = ps_ud.tile([1, TQ], F32, tag="ps_ud")
            for i in range(NKT):
                S0 = ps_s.tile([128, TQ], F32, tag="ps_s")
                nc.tensor.matmul(S0, khT[0:64, 128 * i:128 * (i + 1)], qhT[h][0:64, :],
                                 start=True, stop=True)
                S1 = ps_s.tile([128, TQ], F32, tag="ps_s")
                nc.tensor.matmul(S1, khT[64:128, 128 * i:128 * (i + 1)], qhT[h][64:128, :],
                                 start=True, stop=True)
                e0 = expp.tile([128, TQ], F16, tag="exp0")
                nc.scalar.activation(out=e0, in_=S0, func=AF.Exp)
                e1 = expp.tile([128, TQ], F16, tag="exp1")
                nc.scalar.activation(out=e1, in_=S1, func=AF.Exp)
                if debug and h == 0 and i == 0:
                    nc.sync.dma_start(out=dbg["e00"][:, 0:TQ], in_=e0)
                vsl = vh_h[:, i, :]
                nc.tensor.matmul(U0, vsl, e0, start=(i == 0), stop=(i == NKT - 1))
                nc.tensor.matmul(U1, vsl, e1, start=(i == 0), stop=(i == NKT - 1))
                nc.tensor.matmul(den0, ones_col, e0, start=(i == 0), stop=(i == NKT - 1))
                nc.tensor.matmul(den1, ones_col, e1, start=(i == 0), stop=(i == NKT - 1))

            invs = []
            for dps in (den0, den1):
                invf = tmpp.tile([1, TQ], F32, tag="tmp")
                scr = tmpp.tile([1, TQ], F32, tag="tmp")
                nc.vector.reciprocal_approx_accurate(out=invf, in_=dps, scratch=scr)
                inv16 = tmpp.tile([1, TQ], F16, tag="tmp")
                nc.vector.tensor_copy(out=inv16, in_=invf)
                invs.append(inv16)
            inv0, inv1 = invs
            pb0 = ps_s.tile([128, TQ], F32, tag="ps_s")
            nc.tensor.matmul(pb0, ones_row, inv0, start=True, stop=True)
            pb1 = ps_s.tile([128, TQ], F32, tag="ps_s")
            nc.tensor.matmul(pb1, negl_row, inv1, start=True, stop=True)
            u0s = tmpp.tile([128, TQ], F32, tag="tmp")
            nc.vector.tensor_copy(out=u0s, in_=U0)
            u1s = tmpp.tile([128, TQ], F32, tag="tmp")
            nc.vector.tensor_copy(out=u1s, in_=U1)
            t1 = tmpp.tile([128, TQ], F32, tag="tmp")
            nc.vector.tensor_tensor(t1, u0s, pb0, OP.mult)
            t2 = tmpp.tile([128, TQ], F32, tag="tmp")
            nc.vector.tensor_tensor(t2, u1s, pb1, OP.mult)
            t3 = tmpp.tile([128, TQ], F32, tag="tmp")
            nc.gpsimd.tensor_tensor(t3, t1, t2, OP.add)
            at = pp.tile([128, TQ], F16, tag=f"attnT{h}")
            nc.vector.tensor_scalar_add(at, t3, bvl_sb[:, h:h + 1])
            attnT.append(at)
            if debug:
                nc.sync.dma_start(out=dbg["att"][h], in_=at)
                if h == 0:
                    nc.sync.dma_start(out=dbg["khT0"], in_=khT)
                    nc.sync.dma_start(out=dbg["vh0"], in_=vhg[:, 0, :])

        # ---- LN stats ----
        pssum = ps_ud.tile([1, TQ], F32, tag="ps_ud")
        for h in range(HP):
            nc.tensor.matmul(pssum, ones_col, attnT[h], start=(h == 0), stop=(h == HP - 1))
        pssq = ps_ud.tile([1, TQ], F32, tag="ps_ud")
        for h in range(HP):
            sq = tmpp.tile([128, TQ], F16, tag="tmp")
            nc.vector.tensor_tensor(sq, attnT[h], attnT[h], OP.mult)
            nc.tensor.matmul(pssq, ones_col, sq, start=(h == 0), stop=(h == HP - 1))

        mu = rowp.tile([1, TQ], F32, tag="row")
        nc.vector.tensor_scalar_mul(mu, pssum, 1.0 / C)
        e2 = rowp.tile([1, TQ], F32, tag="row")
        nc.vector.tensor_scalar_mul(e2, pssq, 1.0 / C)
        musq = rowp.tile([1, TQ], F32, tag="row")
        nc.vector.tensor_tensor(musq, mu, mu, OP.mult)
        nc.vector.tensor_tensor(e2, e2, musq, OP.subtract)       # var (in-place)
        nc.scalar.activation(out=musq, in_=e2, func=AF.Sqrt, bias=eps_sb)  # std
        nc.vector.reciprocal(out=e2, in_=musq)                   # rstd
        a_row = pp.tile([1, TQ], F32, tag="a_row")
        nc.vector.tensor_scalar_mul(a_row, e2, 1.0 - LAMBDA_INIT)
        # y = a * (attn@Wo2 - mu (x) srow + (1/a) (x) borow); the last two are
        # rank-1 terms folded into the z accumulation so one per-row scale
        # finishes the LayerNorm + bias exactly.
        negmu_row = pp.tile([1, TQ], F16, tag="negmu")
        nc.vector.tensor_scalar_mul(negmu_row, mu, -1.0)
        inva_f = rowp.tile([1, TQ], F32, tag="row")
        scr_f = rowp.tile([1, TQ], F32, tag="row")
        nc.vector.reciprocal_approx_accurate(out=inva_f, in_=a_row, scratch=scr_f)
        inva_row = pp.tile([1, TQ], F16, tag="inva")
        nc.vector.tensor_copy(out=inva_row, in_=inva_f)
        if debug:
            nc.sync.dma_start(out=dbg["ab"][0:1], in_=a_row)
            nc.sync.dma_start(out=dbg["ab"][1:2], in_=a_row)

        a_col = pp.tile([128, 4], F32, tag="a_col")
        for t in range(4):
            pt = ps_ud.tile([128, 1], F32, tag="ps_ud")
            nc.tensor.transpose(pt, a_row[:, 128 * t:128 * (t + 1)], ident1)
            nc.vector.tensor_copy(out=a_col[:, t:t + 1], in_=pt)

        # ---- output projection + fixup ----
        wo_sb = wxp.tile([128, HP, M], F16, tag="wx")
        nc.sync.dma_start(out=wo_sb, in_=wo_d.rearrange("(a p) m -> p a m", p=128))
        for t in range(4):
            pz = ps_s.tile([128, M], F32, tag="ps_s")
            for h in range(HP):
                nc.tensor.matmul(pz, attnT[h][:, 128 * t:128 * (t + 1)], wo_sb[:, h, :],
                                 start=(h == 0), stop=False)
            nc.tensor.matmul(pz, negmu_row[:, 128 * t:128 * (t + 1)], srow_sb, start=False, stop=False)
            nc.tensor.matmul(pz, inva_row[:, 128 * t:128 * (t + 1)], borow_sb, start=False, stop=True)
            y_sb = yp.tile([128, M], F32, tag="y")
            nc.vector.tensor_scalar_mul(y_sb, pz, a_col[:, t:t + 1])
            nc.sync.dma_start(out=y_d[128 * t:128 * (t + 1), :], in_=y_sb)

    nc.compile()
    return nc


def _prep_inputs(inputs):
    """Host-side prep shared by all cores; returns (lam, shared dict)."""
    f32 = np.float32
    q = np.asarray(inputs["query"], f32)
    k = np.asarray(inputs["key"], f32)
    v = np.asarray(inputs["value"], f32)
    Wq = np.asarray(inputs["Wq"], f32)
    Wk = np.asarray(inputs["Wk"], f32)
    Wv = np.asarray(inputs["Wv"], f32)
    Wo = np.asarray(inputs["Wo"], f32)
    bq = np.asarray(inputs["bq"], f32)
    bk = np.asarray(inputs["bk"], f32)
    bv = np.asarray(inputs["bv"], f32)
    bo = np.asarray(inputs["bo"], f32)
    lq1 = np.asarray(inputs["lq1"], f32)
    lk1 = np.asarray(inputs["lk1"], f32)
    lq2 = np.asarray(inputs["lq2"], f32)
    lk2 = np.asarray(inputs["lk2"], f32)
    ln_scale = np.asarray(inputs["ln_scale"], f32)
    ln_bias = np.asarray(inputs["ln_bias"], f32)

    lam = float(np.exp(np.sum(lq1 * lk1, dtype=f32)) - np.exp(np.sum(lq2 * lk2, dtype=f32)) + LAMBDA_INIT)

    f16 = np.float16
    Wo2 = (ln_scale[:, None] * Wo).astype(f32)
    shared = {
        "wq": np.ascontiguousarray((Wq * SCALE).astype(f16)),
        "wk": np.ascontiguousarray(Wk.astype(f16)),
        "wv": np.ascontiguousarray(Wv.astype(f16)),
        "wo": np.ascontiguousarray(Wo2.astype(f16)),
        "bq": np.ascontiguousarray((bq * SCALE).reshape(HP, 128).T),
        "bk": np.ascontiguousarray(bk.reshape(HP, 128).T),
        "bvl": np.ascontiguousarray(((1.0 - lam) * bv).reshape(HP, 128).T),
        "srow": np.ascontiguousarray(Wo2.sum(axis=0, dtype=f32)[None, :].astype(f16)),
        "borow": np.ascontiguousarray(
            ((1.0 - LAMBDA_INIT) * (ln_bias @ Wo) + bo)[None, :].astype(f16)),
        "ones": np.ones((128, 1), f16),
        "onesr": np.ones((1, 128), f16),
        "negl": np.full((1, 128), -lam, f16),
    }
    return lam, q, k, v, shared


def _make_in_maps(q, k, v, shared):
    in_maps = []
    for c in range(8):
        b, ch = c // 4, c % 4
        in_maps.append(dict(
            shared,
            qt=np.ascontiguousarray(q[b, ch * TQ:(ch + 1) * TQ, :].T.astype(np.float16)),
            kt=np.ascontiguousarray(k[b].T.astype(np.float16)),
            vt=np.ascontiguousarray(v[b].T.astype(np.float16)),
        ))
    return in_maps


def kernel(**inputs) -> np.ndarray:
    from concourse import bass_utils

    lam, q, k, v, shared = _prep_inputs(inputs)
    key = round(lam, 6)
    if key not in _CACHE:
        _CACHE[key] = _build(lam)
    nc = _CACHE[key]

    in_maps = _make_in_maps(q, k, v, shared)
    res = bass_utils.run_bass_kernel_spmd(nc, in_maps, core_ids=list(range(8)))
    out = np.empty((B, T, M), np.float32)
    for c in range(8):
        b, ch = c // 4, c % 4
        out[b, ch * TQ:(ch + 1) * TQ, :] = res.results[c]["y"]
    return out


# revision 18
# speedup vs baseline: 1.8621x; 1.4022x over previous
"""MultiHeadDiffAttention Trainium2 Bass kernel.

Strategy: data-parallel over (batch, query-row-chunk). 8 cores, each runs the
same program on different data: core c handles batch b = c // 4 and query rows
[(c % 4) * 512, (c % 4 + 1) * 512). No collectives needed — softmax is over
keys (fully on-core) and LayerNorm is per-token (fully on-core).

Per-core dataflow (all matmuls fp32r: full-rate 4-byte PE mode, ~1.5e-4 rel):
  - host pre-transposes q/k/v to [D, T] layout (pure data movement)
  - qhT[j] = (Wq/8).T @ qT + bq/8          [1024, 512]   (8 j-tiles of 128)
  - per head-pair h (2 doubled heads j0/j1 = partitions 0-63 / 64-127):
      khT_h = Wk_h.T @ kT + bk             [128, 2048]
      vh_h  = vT.T @ Wv_h                  [2048, 128]   (2-pair groups)
      S_j^T = khT_j.T @ qhT_j              [2048, 512]   (K=64 matmuls)
      P_j   = exp(S_j^T)                   (no max-subtraction: |S| <~ 2)
      U_j^T = vh_h.T @ P_j    (PSUM accum over 16 key tiles)   [128, 512]
      den_j = 1^T @ P_j       (PSUM accum)                     [1, 512]
      attnT_h = U0/den0 - lam*U1/den1 + (1-lam)*bv_h           [128, 512]
  - LayerNorm folded into output projection:
      z = attn @ (ln_scale*Wo);  stats via ones-matmuls over attnT
      y = (1-LI)*rstd*(z - mu*colsum(Wo2)) + ((1-LI)*ln_bias@Wo + bo)
"""

import sys

if "/opt/trn_rl_repo" not in sys.path:
    sys.path.insert(0, "/opt/trn_rl_repo")

from contextlib import ExitStack

import numpy as np

B, T, D, H2, DH, M = 2, 2048, 512, 16, 64, 512
HP = 8          # head pairs
C = H2 * DH     # 1024 projected channels
TQ = 512        # query rows per core
NKT = T // 128  # 16 key tiles
LAMBDA_INIT = 0.8 - 0.6 * float(np.exp(-0.3 * 2))
SCALE = 1.0 / float(np.sqrt(DH))
EPS = 1e-6

_CACHE = {}


def _build(lam: float, debug: bool = False):
    import concourse.mybir as mybir
    import concourse.tile as tile
    from concourse import bacc
    from concourse.masks import make_identity

    F32 = mybir.dt.float32
    F32R = mybir.dt.float32r
    F16 = mybir.dt.float16
    AF = mybir.ActivationFunctionType
    OP = mybir.AluOpType

    nc = bacc.Bacc("TRN2", target_bir_lowering=False, debug=False)
    dt_in = lambda n, s: nc.dram_tensor(n, s, F32, kind="ExternalInput").ap()
    dt16 = lambda n, s: nc.dram_tensor(n, s, F16, kind="ExternalInput").ap()
    qT_d = dt16("qt", [D, TQ])
    kT_d = dt16("kt", [D, T])
    vT_d = dt16("vt", [D, T])
    wq_d = dt16("wq", [D, C])
    wk_d = dt16("wk", [D, C])
    wv_d = dt16("wv", [D, C])
    wo_d = dt16("wo", [C, M])
    bq_d = dt_in("bq", [128, HP])
    bk_d = dt_in("bk", [128, HP])
    bvl_d = dt_in("bvl", [128, HP])
    srow_d = dt16("srow", [1, M])
    borow_d = dt16("borow", [1, M])
    ones_d = dt16("ones", [128, 1])
    onesr_d = dt16("onesr", [1, 128])
    negl_d = dt16("negl", [1, 128])
    y_d = nc.dram_tensor("y", [TQ, M], F32, kind="ExternalOutput").ap()
    dbg = {}
    if debug:
        dbg["qhT0"] = nc.dram_tensor("dbg_qhT0", [128, TQ], F16, kind="ExternalOutput").ap()
        dbg["khT0"] = nc.dram_tensor("dbg_khT0", [128, T], F16, kind="ExternalOutput").ap()
        dbg["vh0"] = nc.dram_tensor("dbg_vh0", [128, 256], F16, kind="ExternalOutput").ap()
        dbg["att"] = nc.dram_tensor("dbg_att", [HP, 128, TQ], F16, kind="ExternalOutput").ap()
        dbg["den"] = nc.dram_tensor("dbg_den", [2 * HP, TQ], F32, kind="ExternalOutput").ap()
        dbg["ab"] = nc.dram_tensor("dbg_ab", [2, TQ], F32, kind="ExternalOutput").ap()
        dbg["e00"] = nc.dram_tensor("dbg_e00", [128, 1024], F16, kind="ExternalOutput").ap()

    with tile.TileContext(nc) as tc, ExitStack() as ctx:
        pp = ctx.enter_context(tc.tile_pool(name="persist", bufs=1))
        khp = ctx.enter_context(tc.tile_pool(name="khp", bufs=2))
        vhp = ctx.enter_context(tc.tile_pool(name="vhp", bufs=2))
        wxp = ctx.enter_context(tc.tile_pool(name="wxp", bufs=1))
        wkp = ctx.enter_context(tc.tile_pool(name="wkp", bufs=2))
        wvp = ctx.enter_context(tc.tile_pool(name="wvp", bufs=2))
        expp = ctx.enter_context(tc.tile_pool(name="expp", bufs=2))
        tmpp = ctx.enter_context(tc.tile_pool(name="tmpp", bufs=6))
        rowp = ctx.enter_context(tc.tile_pool(name="rowp", bufs=3))
        yp = ctx.enter_context(tc.tile_pool(name="yp", bufs=2))
        ps_s = ctx.enter_context(tc.tile_pool(name="ps_s", bufs=3, space="PSUM"))
        ps_u = ctx.enter_context(tc.tile_pool(name="ps_u", bufs=2, space="PSUM"))

        # ---- constants ----
        ident1 = pp.tile([1, 1], F32, tag="ident1")
        nc.vector.memset(ident1, 1.0)
        ones_col = pp.tile([128, 1], F16, tag="ones_col")
        nc.sync.dma_start(out=ones_col, in_=ones_d)
        ones_row = pp.tile([1, 128], F16, tag="ones_row")
        nc.sync.dma_start(out=ones_row, in_=onesr_d)
        negl_row = pp.tile([1, 128], F16, tag="negl_row")
        nc.sync.dma_start(out=negl_row, in_=negl_d)
        bq_sb = pp.tile([128, HP], F32, tag="bq")
        nc.sync.dma_start(out=bq_sb, in_=bq_d)
        bk_sb = pp.tile([128, HP], F32, tag="bk")
        nc.sync.dma_start(out=bk_sb, in_=bk_d)
        bvl_sb = pp.tile([128, HP], F32, tag="bvl")
        nc.sync.dma_start(out=bvl_sb, in_=bvl_d)
        srow_sb = pp.tile([1, M], F16, tag="srow")
        nc.sync.dma_start(out=srow_sb, in_=srow_d)
        borow_sb = pp.tile([1, M], F16, tag="borow")
        nc.sync.dma_start(out=borow_sb, in_=borow_d)
        eps_sb = pp.tile([1, 1], F32, tag="eps")
        nc.vector.memset(eps_sb, EPS)

        # ---- persistent transposed inputs (q first: unblocks first matmuls) ----
        qTt = pp.tile([128, 4, TQ], F16, tag="qT")
        nc.sync.dma_start(out=qTt, in_=qT_d.rearrange("(a p) t -> p a t", p=128))

        # ---- q projection: qhT[j] [128, 512] ----
        wq_sb = wxp.tile([128, 4, C], F16, tag="wx")
        nc.sync.dma_start(out=wq_sb, in_=wq_d.rearrange("(a p) n -> p a n", p=128))
        qhT = []
        for j in range(HP):
            pq = ps_s.tile([128, TQ], F32, tag="ps_s")
            for a in range(4):
                nc.tensor.matmul(pq, wq_sb[:, a, 128 * j:128 * (j + 1)], qTt[:, a, :],
                                 start=(a == 0), stop=(a == 3))
            t = pp.tile([128, TQ], F16, tag=f"qhT{j}")
            nc.vector.tensor_scalar_add(t, pq, bq_sb[:, j:j + 1])
            qhT.append(t)
            if debug and j == 0:
                nc.sync.dma_start(out=dbg["qhT0"], in_=t.bitcast(F32))

        kT = pp.tile([128, 4, T], F16, tag="kT")
        for a in range(4):
            nc.sync.dma_start(out=kT[:, a, :], in_=kT_d.rearrange("(a p) t -> p a t", p=128)[:, a, :])
        vT = pp.tile([128, 4, T], F16, tag="vT")
        for a in range(4):
            nc.sync.dma_start(out=vT[:, a, :], in_=vT_d.rearrange("(a p) t -> p a t", p=128)[:, a, :])

        # ---- pair loop ----
        attnT = []
        for h in range(HP):
            # vh for a 2-pair group (pairs h, h+1) when h even
            if h % 2 == 0:
                wv_sl = wvp.tile([128, 4, 256], F16, tag="wv")
                nc.sync.dma_start(
                    out=wv_sl,
                    in_=wv_d.rearrange("(a p) n -> p a n", p=128)[:, :, 256 * (h // 2):256 * (h // 2 + 1)])
                vhg = vhp.tile([128, NKT, 256], F16, tag="vhg")
                for i in range(NKT):
                    pv = ps_s.tile([128, 256], F32, tag="ps_s")
                    for a in range(4):
                        nc.tensor.matmul(pv, vT[:, a, 128 * i:128 * (i + 1)], wv_sl[:, a, :],
                                         start=(a == 0), stop=(a == 3))
                    nc.vector.tensor_copy(out=vhg[:, i, :], in_=pv)
            vh_h = vhg[:, :, 128 * (h % 2):128 * (h % 2) + 128]

            # khT_h [128, 2048]
            wk_sl = wkp.tile([128, 4, 128], F16, tag="wk")
            nc.sync.dma_start(
                out=wk_sl,
                in_=wk_d.rearrange("(a p) n -> p a n", p=128)[:, :, 128 * h:128 * (h + 1)])
            khT = khp.tile([128, T], F16, tag="khT")
            for cch in range(4):
                pk = ps_s.tile([128, 512], F32, tag="ps_s")
                for a in range(4):
                    nc.tensor.matmul(pk, wk_sl[:, a, :], kT[:, a, 512 * cch:512 * (cch + 1)],
                                     start=(a == 0), stop=(a == 3))
                nc.vector.tensor_scalar_add(khT[:, 512 * cch:512 * (cch + 1)], pk, bk_sb[:, h:h + 1])

            # attention: 8 groups of 2 key tiles
            U0 = ps_u.tile([128, TQ], F32, tag="ps_u")
            U1 = ps_u.tile([128, TQ], F32, tag="ps_u")
            e0a = expp.tile([128, NKT, TQ], F16, tag="exp0")
            e1a = expp.tile([128, NKT, TQ], F16, tag="exp1")
            for g in range(NKT // 2):
                S0 = ps_s.tile([128, 1024], F32, tag="ps_s")
                S1 = ps_s.tile([128, 1024], F32, tag="ps_s")
                for st in range(2):
                    i = 2 * g + st
                    nc.tensor.matmul(S0[:, 512 * st:512 * (st + 1)],
                                     khT[0:64, 128 * i:128 * (i + 1)], qhT[h][0:64, :],
                                     start=True, stop=True)
                    nc.tensor.matmul(S1[:, 512 * st:512 * (st + 1)],
                                     khT[64:128, 128 * i:128 * (i + 1)], qhT[h][64:128, :],
                                     start=True, stop=True)
                nc.scalar.activation(out=e0a[:, 2 * g:2 * g + 2, :], in_=S0, func=AF.Exp)
                nc.scalar.activation(out=e1a[:, 2 * g:2 * g + 2, :], in_=S1, func=AF.Exp)
                for st in range(2):
                    i = 2 * g + st
                    vsl = vh_h[:, i, :]
                    nc.tensor.matmul(U0, vsl, e0a[:, i, :], start=(i == 0), stop=(i == NKT - 1))
                    nc.tensor.matmul(U1, vsl, e1a[:, i, :], start=(i == 0), stop=(i == NKT - 1))
            if debug and h == 0:
                nc.sync.dma_start(out=dbg["e00"], in_=e0a[:, 0:2, :])

            # denominators: col-packed M=1 stream at pair end (4 concurrent
            # 32-col strips), then one ones-matmul folds the 4 partials
            invs = []
            for ea in (e0a, e1a):
                dps = ps_s.tile([128, TQ], F32, tag="ps_s")
                nc.vector.memset(dps, 0.0)
                for i in range(NKT):
                    cg = i % 4
                    nc.tensor.matmul(dps[32 * cg:32 * cg + 1, :], ones_col, ea[:, i, :],
                                     start=(i < 4), stop=(i >= NKT - 4),
                                     tile_position=(0, 32 * cg))
                densb = tmpp.tile([128, TQ], F16, tag="tmp")
                nc.vector.tensor_copy(out=densb, in_=dps)
                dtot = ps_s.tile([1, TQ], F32, tag="ps_s")
                nc.tensor.matmul(dtot, ones_col, densb, start=True, stop=True)
                invf = tmpp.tile([1, TQ], F32, tag="tmp")
                scr = tmpp.tile([1, TQ], F32, tag="tmp")
                nc.vector.reciprocal_approx_accurate(out=invf, in_=dtot, scratch=scr)
                inv16 = tmpp.tile([1, TQ], F16, tag="tmp")
                nc.vector.tensor_copy(out=inv16, in_=invf)
                invs.append(inv16)
            inv0, inv1 = invs
            pb0 = ps_s.tile([128, TQ], F32, tag="ps_s")
            nc.tensor.matmul(pb0, ones_row, inv0, start=True, stop=True)
            pb1 = ps_s.tile([128, TQ], F32, tag="ps_s")
            nc.tensor.matmul(pb1, negl_row, inv1, start=True, stop=True)
            u0s = tmpp.tile([128, TQ], F32, tag="tmp")
            nc.vector.tensor_copy(out=u0s, in_=U0)
            u1s = tmpp.tile([128, TQ], F32, tag="tmp")
            nc.vector.tensor_copy(out=u1s, in_=U1)
            t1 = tmpp.tile([128, TQ], F32, tag="tmp")
            nc.vector.tensor_tensor(t1, u0s, pb0, OP.mult)
            t2 = tmpp.tile([128, TQ], F32, tag="tmp")
            nc.vector.tensor_tensor(t2, u1s, pb1, OP.mult)
            t3 = tmpp.tile([128, TQ], F32, tag="tmp")
            nc.gpsimd.tensor_tensor(t3, t1, t2, OP.add)
            at = pp.tile([128, TQ], F16, tag=f"attnT{h}")
            nc.vector.tensor_scalar_add(at, t3, bvl_sb[:, h:h + 1])
            attnT.append(at)
            if debug:
                nc.sync.dma_start(out=dbg["att"][h], in_=at)
                if h == 0:
                    nc.sync.dma_start(out=dbg["khT0"], in_=khT)
                    nc.sync.dma_start(out=dbg["vh0"], in_=vhg[:, 0, :])

        # ---- LN stats ----
        pssum = ps_u.tile([1, TQ], F32, tag="ps_u")
        for h in range(HP):
            nc.tensor.matmul(pssum, ones_col, attnT[h], start=(h == 0), stop=(h == HP - 1))
        pssq = ps_u.tile([1, TQ], F32, tag="ps_u")
        for h in range(HP):
            sq = tmpp.tile([128, TQ], F16, tag="tmp")
            nc.vector.tensor_tensor(sq, attnT[h], attnT[h], OP.mult)
            nc.tensor.matmul(pssq, ones_col, sq, start=(h == 0), stop=(h == HP - 1))

        mu = rowp.tile([1, TQ], F32, tag="row")
        nc.vector.tensor_scalar_mul(mu, pssum, 1.0 / C)
        e2 = rowp.tile([1, TQ], F32, tag="row")
        nc.vector.tensor_scalar_mul(e2, pssq, 1.0 / C)
        musq = rowp.tile([1, TQ], F32, tag="row")
        nc.vector.tensor_tensor(musq, mu, mu, OP.mult)
        nc.vector.tensor_tensor(e2, e2, musq, OP.subtract)       # var (in-place)
        nc.scalar.activation(out=musq, in_=e2, func=AF.Sqrt, bias=eps_sb)  # std
        nc.vector.reciprocal(out=e2, in_=musq)                   # rstd
        a_row = pp.tile([1, TQ], F32, tag="a_row")
        nc.vector.tensor_scalar_mul(a_row, e2, 1.0 - LAMBDA_INIT)
        # y = a * (attn@Wo2 - mu (x) srow + (1/a) (x) borow); the last two are
        # rank-1 terms folded into the z accumulation so one per-row scale
        # finishes the LayerNorm + bias exactly.
        negmu_row = pp.tile([1, TQ], F16, tag="negmu")
        nc.vector.tensor_scalar_mul(negmu_row, mu, -1.0)
        inva_f = rowp.tile([1, TQ], F32, tag="row")
        scr_f = rowp.tile([1, TQ], F32, tag="row")
        nc.vector.reciprocal_approx_accurate(out=inva_f, in_=a_row, scratch=scr_f)
        inva_row = pp.tile([1, TQ], F16, tag="inva")
        nc.vector.tensor_copy(out=inva_row, in_=inva_f)
        if debug:
            nc.sync.dma_start(out=dbg["ab"][0:1], in_=a_row)
            nc.sync.dma_start(out=dbg["ab"][1:2], in_=a_row)

        a_col = pp.tile([128, 4], F32, tag="a_col")
        for t in range(4):
            pt = ps_u.tile([128, 1], F32, tag="ps_u")
            nc.tensor.transpose(pt, a_row[:, 128 * t:128 * (t + 1)], ident1)
            nc.vector.tensor_copy(out=a_col[:, t:t + 1], in_=pt)

        # ---- output projection + fixup ----
        wo_sb = wxp.tile([128, HP, M], F16, tag="wx")
        nc.sync.dma_start(out=wo_sb, in_=wo_d.rearrange("(a p) m -> p a m", p=128))
        for t in range(4):
            pz = ps_s.tile([128, M], F32, tag="ps_s")
            for h in range(HP):
                nc.tensor.matmul(pz, attnT[h][:, 128 * t:128 * (t + 1)], wo_sb[:, h, :],
                                 start=(h == 0), stop=False)
            nc.tensor.matmul(pz, negmu_row[:, 128 * t:128 * (t + 1)], srow_sb, start=False, stop=False)
            nc.tensor.matmul(pz, inva_row[:, 128 * t:128 * (t + 1)], borow_sb, start=False, stop=True)
            y_sb = yp.tile([128, M], F32, tag="y")
            nc.vector.tensor_scalar_mul(y_sb, pz, a_col[:, t:t + 1])
            nc.sync.dma_start(out=y_d[128 * t:128 * (t + 1), :], in_=y_sb)

    nc.compile()
    return nc


def _prep_inputs(inputs):
    """Host-side prep shared by all cores; returns (lam, shared dict)."""
    f32 = np.float32
    q = np.asarray(inputs["query"], f32)
    k = np.asarray(inputs["key"], f32)
    v = np.asarray(inputs["value"], f32)
    Wq = np.asarray(inputs["Wq"], f32)
    Wk = np.asarray(inputs["Wk"], f32)
    Wv = np.asarray(inputs["Wv"], f32)
    Wo = np.asarray(inputs["Wo"], f32)
    bq = np.asarray(inputs["bq"], f32)
    bk = np.asarray(inputs["bk"], f32)
    bv = np.asarray(inputs["bv"], f32)
    bo = np.asarray(inputs["bo"], f32)
    lq1 = np.asarray(inputs["lq1"], f32)
    lk1 = np.asarray(inputs["lk1"], f32)
    lq2 = np.asarray(inputs["lq2"], f32)
    lk2 = np.asarray(inputs["lk2"], f32)
    ln_scale = np.asarray(inputs["ln_scale"], f32)
    ln_bias = np.asarray(inputs["ln_bias"], f32)

    lam = float(np.exp(np.sum(lq1 * lk1, dtype=f32)) - np.exp(np.sum(lq2 * lk2, dtype=f32)) + LAMBDA_INIT)

    f16 = np.float16
    Wo2 = (ln_scale[:, None] * Wo).astype(f32)
    shared = {
        "wq": np.ascontiguousarray((Wq * SCALE).astype(f16)),
        "wk": np.ascontiguousarray(Wk.astype(f16)),
        "wv": np.ascontiguousarray(Wv.astype(f16)),
        "wo": np.ascontiguousarray(Wo2.astype(f16)),
        "bq": np.ascontiguousarray((bq * SCALE).reshape(HP, 128).T),
        "bk": np.ascontiguousarray(bk.reshape(HP, 128).T),
        "bvl": np.ascontiguousarray(((1.0 - lam) * bv).reshape(HP, 128).T),
        "srow": np.ascontiguousarray(Wo2.sum(axis=0, dtype=f32)[None, :].astype(f16)),
        "borow": np.ascontiguousarray(
            ((1.0 - LAMBDA_INIT) * (ln_bias @ Wo) + bo)[None, :].astype(f16)),
        "ones": np.ones((128, 1), f16),
        "onesr": np.ones((1, 128), f16),
        "negl": np.full((1, 128), -lam, f16),
    }
    return lam, q, k, v, shared


def _make_in_maps(q, k, v, shared):
    in_maps = []
    for c in range(8):
        b, ch = c // 4, c % 4
        in_maps.append(dict(
            shared,
            qt=np.ascontiguousarray(q[b, ch * TQ:(ch + 1) * TQ, :].T.astype(np.float16)),
            kt=np.ascontiguousarray(k[b].T.astype(np.float16)),
            vt=np.ascontiguousarray(v[b].T.astype(np.float16)),
        ))
    return in_maps


def kernel(**inputs) -> np.ndarray:
    from concourse import bass_utils

    lam, q, k, v, shared = _prep_inputs(inputs)
    key = round(lam, 6)
    if key not in _CACHE:
        _CACHE[key] = _build(lam)
    nc = _CACHE[key]

    in_maps = _make_in_maps(q, k, v, shared)
    res = bass_utils.run_bass_kernel_spmd(nc, in_maps, core_ids=list(range(8)))
    out = np.empty((B, T, M), np.float32)
    for c in range(8):
        b, ch = c // 4, c % 4
        out[b, ch * TQ:(ch + 1) * TQ, :] = res.results[c]["y"]
    return out
